# revision 11
# baseline (speedup 1.0000x reference)
"""Trainium2 Bass kernel for CRATE-style subspace attention (nn_Attention_37091337568712).

Reference computation (fp32):
    w = x @ Wqkv                    # (b, n, 1024), shared q=k=v projection
    S = (w @ w^T) * d^-0.5          # per head, (b, h, n, n)
    attn = softmax(S, axis=-1) * (1 - mask[:, None, None, :])
    out = attn @ w ; y = out @ Wout + bout

Sharding: 8 cores = 2 batches x 4 head-groups (4 heads each, 2 pairs of 2).

fp8 design (vs the 204us bf16 baseline):
  - The gram-matrix diagonal S_ii = |w_i|^2/8 ~ 8..15 dominates softmax and
    cannot survive fp8 E. Diagonal extraction: S' = S - 192*I via a small
    accumulating identity matmul, so exp(S') underflows fp8 to exact 0 on the
    diagonal; the exact diagonal term g_i = exp(SCALE*s_ii - SH*ln2) is
    computed separately (w^2 ones-matmul + one ACT exp on a spread layout)
    and merged into numerator/denominator at scale time.
  - With the diagonal gone, logits fit fp8: S via DoubleRow fp8 matmuls
    (w as [32, 2kt, .] row-banded per head, 2 heads co-executing), E in fp8
    (ACT: true exp w/ -SH*ln2 bias; DVE/Pool: Schraudolph uint8 bit trick,
    negative bits saturate to +0), V' in fp8, AV as DoubleRow over jc pairs.
    PE attention work drops ~2.6x; exp is split over ACT+DVE+Pool.
  - softmax denominator via unmasked ones columns in the AV stationary (M=65).
  - numerator merge: osT = rb*raw + zb*wm (rb=1/den', zb=g/den' broadcast
    rows; wm = masked w kept per-head at partitions 0..63).
  - numpy-sim of this exact pipeline: rel err ~5.5e-3 (gate 2e-2).
"""

import sys

if "/opt/trn_rl_repo" not in sys.path:
    sys.path.insert(0, "/opt/trn_rl_repo")

import numpy as np
import ml_dtypes

import concourse.bass as bass
import concourse.mybir as mybir
from concourse import masks
from concourse.bass_utils import run_bass_kernel_spmd
from concourse.tile import TileContext

FP = mybir.dt.float32
I32 = mybir.dt.int32
I16 = mybir.dt.int16
U8 = mybir.dt.uint8
BF = mybir.dt.bfloat16
F8 = mybir.dt.float8e4
BF_NP = ml_dtypes.bfloat16
DR = mybir.MatmulPerfMode.DoubleRow


def _split_multiwaits(bir_json: bytes) -> bytes:
    """Split multi-wait instructions for this container's walrus (single
    sync wait per instruction)."""
    import json

    bir = json.loads(bir_json)
    changed = False
    for fn in bir.get("functions", []):
        for bb in fn.get("blocks", []):
            insts = bb.get("instructions")
            if insts is None:
                continue
            new_insts = []
            for ins in insts:
                si = ins.get("sync_info")
                waits = si.get("on_wait") if si else None
                if waits and len(waits) > 1:
                    changed = True
                    for wi, w in enumerate(waits[:-1]):
                        new_insts.append({
                            "name": f"{ins['name']}_w{wi}",
                            "opcode": "EventSemaphore",
                            "engine": ins["engine"],
                            "ins": [],
                            "outs": [],
                            "debug": ins.get("debug", 0),
                            "sync_info": {"on_wait": [w], "on_update": []},
                        })
                    si["on_wait"] = [waits[-1]]
                new_insts.append(ins)
            bb["instructions"] = new_insts
    if not changed:
        return bir_json
    return json.dumps(bir).encode()


def _install_bir_legalizer():
    from concourse import bass2jax, bass_utils

    if getattr(bass2jax, "_multiwait_legalizer_installed", False):
        return
    orig = bass_utils.compile_bir_kernel

    def wrapped(bir_json, tmpdir, neff_name="file.neff"):
        try:
            return orig(_split_multiwaits(bytes(bir_json)), tmpdir, neff_name)
        except BaseException as e:
            import subprocess, traceback
            try:
                with open("/tmp/bass_compile_err.txt", "w") as f:
                    traceback.print_exc(file=f)
                    ee = e
                    while ee is not None:
                        if isinstance(ee, subprocess.CalledProcessError):
                            out = ee.stdout or ""
                            if isinstance(out, bytes):
                                out = out.decode(errors="replace")
                            f.write("\n==WALRUS STDOUT (tail)==\n" + out[-12000:])
                        ee = ee.__cause__ or ee.__context__
            except Exception:
                pass
            raise

    bass2jax.compile_bir_kernel = wrapped
    bass2jax._multiwait_legalizer_installed = True


N = 2048          # sequence length
DIM = 1024        # model dim
DH = 64           # head dim
EC = 256          # local inner columns (4 heads)
KC = DIM // 128   # 8 contraction chunks for the projection
PAIRS = 2         # head pairs per core (2 heads stacked on 128 partitions)
SCALE = DH ** -0.5
LN2 = float(np.log(2.0))

SH = 2.0          # global log2 shift of E (softmax-invariant)
BDIAG = 192.0     # diagonal extraction; exp(S'-ii) underflows fp8 to 0
C8 = 0.0434       # Schraudolph tuning constant
A8 = 8.0 * (1.0 / LN2) * SCALE        # uint8 Schraudolph: bits = A8*S + B8
B8 = 8.0 * (7.0 - C8 - SH)
C8P = 0.0434      # Pool variant (retune if Pool rounding differs)
B8P = 8.0 * (7.0 - C8P - SH)

EXPF = mybir.ActivationFunctionType.Exp
COPYF = mybir.ActivationFunctionType.Copy

# exp engine split: 'A' = ACT true exp->fp8, 'D' = DVE Schraudolph uint8,
# 'P' = Pool Schraudolph uint8. POOL_EXP gate for bring-up.
POOL_EXP = True
ACT_FP8_OUT = True

_program_cache = {}


def _exp_schedule():
    """Greedy finish-time assignment of the 128 exp units to ACT/DVE.
    (GPSIMD cannot read PSUM, so Pool does SBUF-side work instead.)"""
    cost = {"A": 996.0, "D": 1192.0}
    load = {"A": 7000.0, "D": 15000.0}  # fixed-work seeds
    sched = []
    for _ in range(128):
        e = min(cost, key=lambda k: load[k] + cost[k])
        sched.append(e)
        load[e] += cost[e]
    return sched


def build_program():
    nc = bass.Bass()

    xT = nc.declare_dram_parameter("xT", [DIM, N], BF, isOutput=False)
    wqkv = nc.declare_dram_parameter("wqkv", [128, KC, EC], BF, isOutput=False)
    wout = nc.declare_dram_parameter("wout", [128, PAIRS, DIM], BF, isOutput=False)
    mask_d = nc.declare_dram_parameter("mask", [N], I32, isOutput=False)
    mf_d = nc.declare_dram_parameter("mf", [N], BF, isOutput=False)
    y = nc.declare_dram_parameter("y", [N, DIM], BF, isOutput=True)
    # DRAM scratch: broadcast rows (ch: 0=rb, 1=zb) and the fp8 w bounce
    dscr = nc.declare_dram_parameter("dscr", [2, PAIRS, 2, 2, N // 2], BF,
                                     isOutput=True)
    wbn = nc.declare_dram_parameter("wbn", [PAIRS, 128, N], F8, isOutput=True)
    gdr = nc.declare_dram_parameter("gdr", [PAIRS, 2, N], BF, isOutput=True)

    sched = _exp_schedule()
    expi = [0]  # rotating index into sched

    with TileContext(nc) as tc:
        with (
            tc.tile_pool(name="const", bufs=1) as constp,
            tc.tile_pool(name="wts", bufs=1) as wts,
            tc.tile_pool(name="persist", bufs=1) as persist,
            tc.tile_pool(name="xin", bufs=5) as xin,
            tc.tile_pool(name="epool", bufs=6) as epool,
            tc.tile_pool(name="rawp", bufs=3) as rawp,
            tc.tile_pool(name="trpool", bufs=2) as trpool,
            tc.tile_pool(name="mrgp", bufs=3) as mrgp,
            tc.tile_pool(name="ysb", bufs=2) as ysbp,
            tc.tile_pool(name="bcp", bufs=2) as bcp,
        ):
            # ---- weights + input stream first so the projection starts ASAP
            wq_sb = wts.tile([128, KC, EC], BF)
            nc.sync.dma_start(wq_sb[:], wqkv[:])
            xts = []
            for kc in range(KC):
                xt = xin.tile([128, N], BF, name="xt")
                nc.sync.dma_start(xt[:], xT[kc * 128:(kc + 1) * 128, :])
                xts.append(xt)

            # ---- small inputs + mask broadcast on the scalar queue ----
            mask_i = constp.tile([16, 128], I32)
            nc.scalar.dma_start(mask_i[:], mask_d.rearrange("(a b) -> a b", a=16))
            mbc = persist.tile([128, N], BF)       # (1-m) broadcast over rows
            nc.scalar.dma_start(mbc[:], mf_d[:].partition_broadcast(128))
            wout_sb = wts.tile([128, PAIRS, DIM], BF)
            nc.scalar.dma_start(wout_sb[:], wout[:])

            # ---- constants ----
            ident_f = constp.tile([16, 16], FP)
            masks.make_identity(nc, ident_f[:])
            id128 = constp.tile([128, 128], BF)
            masks.make_identity(nc, id128[:])
            negBI = constp.tile([128, 128], BF)
            nc.vector.tensor_scalar_mul(negBI[:], id128[:], -BDIAG)
            expb = constp.tile([128, 1], FP)       # exp bias: -SH*ln2
            nc.gpsimd.memset(expb[:], -SH * LN2)
            hones = constp.tile([128, 2], BF)      # head-membership ones
            nc.gpsimd.memset(hones[:], 0.0)
            nc.gpsimd.memset(hones[0:64, 0:1], 1.0)
            nc.gpsimd.memset(hones[64:128, 1:2], 1.0)
            mask_f = constp.tile([16, 128], FP)
            nc.vector.tensor_scalar(
                out=mask_f[:], in0=mask_i[:], scalar1=-1.0, scalar2=1.0,
                op0=mybir.AluOpType.mult, op1=mybir.AluOpType.add,
            )

            # ---- persistent big tiles ----
            wT2 = persist.tile([128, PAIRS, N], BF)        # [d2, pair, i]
            wf8 = persist.tile([32, PAIRS, 2, 2, N], F8)   # [p32, pair, hh, kt, i]
            w8t = persist.tile([128, PAIRS, N], F8)        # cast scratch
            wm2 = persist.tile([64, PAIRS, 2, N], BF)      # masked w, head rows 0:64
            wsq = persist.tile([128, PAIRS, N], BF)        # w*w
            v2 = persist.tile([128, PAIRS, 8, 2, 2, 72], F8)  # [j, pair, jcp, kt, hh, d72]
            osT2 = persist.tile([128, PAIRS, N], BF)       # merged attn out
            maskc = persist.tile([128, 16], FP)            # (1-mask) [j%128, jc]
            g_sp = persist.tile([128, PAIRS, 16, 2], BF)   # matmul-layout exp(diag)
            g_nat = persist.tile([128, PAIRS, 2, 16], BF)  # natural spread layout
            den_sp = persist.tile([128, 64], BF)
            recip_sp = persist.tile([128, 64], BF)
            z_sp = persist.tile([128, 64], BF)


            # ---- phase 1: projection ----
            with tc.tile_pool(name="ps_proj", bufs=1, space="PSUM") as ps_proj:
                proj_ps = [ps_proj.tile([128, 1024], FP, name=f"proj{t}", tag=f"proj{t}")
                           for t in range(4)]
                for kc in range(KC):
                    for pair in range(PAIRS):
                        for nb in range(2):
                            for sb in range(2):
                                nc.tensor.matmul(
                                    proj_ps[pair * 2 + nb][:, sb * 512:(sb + 1) * 512],
                                    wq_sb[:, kc, pair * 128:(pair + 1) * 128],
                                    xts[kc][:, nb * 1024 + sb * 512:
                                            nb * 1024 + (sb + 1) * 512],
                                    start=(kc == 0), stop=(kc == KC - 1),
                                )
                for pair in range(PAIRS):
                    for nb in range(2):
                        eng = nc.scalar if nb == 0 else nc.vector
                        eng_copy = (nc.scalar.copy if nb == 0
                                    else nc.vector.tensor_copy)
                        eng_copy(wT2[:, pair, nb * 1024:(nb + 1) * 1024],
                                 proj_ps[pair * 2 + nb][:])

            # ---- mask layout transpose ----
            with tc.tile_pool(name="ps_tr", bufs=1, space="PSUM") as ps_tr:
                mt_ps = ps_tr.tile([128, 16], FP, tag="trm")
                nc.tensor.transpose(mt_ps[:], mask_f[:], ident_f[:])
                nc.vector.tensor_copy(maskc[:], mt_ps[:])

            # ---- per-pair prep: wm/wsq/cast/bounce/transpose/V' ----
            nc.vector.memset(v2[:, :, :, :, :, 64], 1.0)
            for pair in range(PAIRS):
                # masked w, per-head at partitions 0:63 (merge operand)
                for hh in range(2):
                    nc.vector.tensor_tensor(
                        out=wm2[:, pair, hh, :],
                        in0=wT2[hh * 64:(hh + 1) * 64, pair, :],
                        in1=mbc[hh * 64:(hh + 1) * 64, :],
                        op=mybir.AluOpType.mult,
                    )
                # w^2 for the diagonal
                nc.gpsimd.tensor_tensor(
                    out=wsq[:, pair, :], in0=wT2[:, pair, :],
                    in1=wT2[:, pair, :], op=mybir.AluOpType.mult,
                )
                # fp8 cast + DRAM bounce into the row-banded S layout
                nc.gpsimd.tensor_copy(w8t[:, pair, :], wT2[:, pair, :])
                nc.scalar.dma_start(wbn[pair], w8t[:, pair, :])
                for hh in range(2):
                    nc.scalar.dma_start(
                        wf8[:, pair, hh, :, :],
                        wbn[pair, hh * 64:(hh + 1) * 64, :]
                          .rearrange("(kt p) i -> p kt i", p=32),
                    )
                # V' via one batched XBAR transpose + masked fp8 build
                tr2 = trpool.tile([128, 16, 128], BF, name="tr2", tag="tr2")
                nc.sync.dma_start_transpose(tr2[:], wT2[:, pair, :])
                for jc in range(16):
                    vdst = v2[:, pair, jc // 2, jc % 2, :, 0:64]
                    vsrc = tr2[:, jc, :].rearrange("p (h x) -> p h x", h=2)
                    nc.gpsimd.tensor_scalar_mul(vdst, vsrc, maskc[:, jc:jc + 1])

            # ---- diagonal g: ones-matmuls on wsq + one ACT exp per pair ----
            with tc.tile_pool(name="ps_g", bufs=1, space="PSUM") as ps_g:
                for pair in range(PAIRS):
                    g_ps = ps_g.tile([128, 16, 2], FP, name="gps", tag="gps")
                    for c in range(16):
                        nc.tensor.matmul(
                            g_ps[:, c, :],
                            wsq[:, pair, c * 128:(c + 1) * 128],
                            hones[:],
                            start=True, stop=True,
                        )
                    nc.scalar.activation(g_sp[:, pair, :, :], g_ps[:],
                                         EXPF, scale=SCALE, bias=expb[:])
                    for hh in range(2):
                        nc.scalar.dma_start(
                            gdr[pair, hh, :].rearrange("(c p) -> p c", p=128),
                            g_sp[:, pair, :, hh])
                        for ibh in range(2):
                            nc.scalar.dma_start(
                                g_nat[:, pair, hh, ibh * 8:(ibh + 1) * 8],
                                gdr[pair, hh, ibh * 1024:(ibh + 1) * 1024])

            # ---- helper emitters ----
            rawst = {}
            bcasts = {}

            def emit_block_finish(pair, ibh, av_t):
                k0 = (pair * 2 + ibh) * 2
                bc = bcp.tile([64, 2, 2, N // 2], BF, name="bc", tag="bc")
                bcasts[(pair, ibh)] = bc
                tiles = []
                for hh in range(2):
                    k = k0 + hh
                    rs = rawp.tile([65, 1024], BF, name=f"rawst{hh}",
                                   tag=f"rawst{hh}")
                    if hh == 0:
                        nc.scalar.copy(rs[:], av_t[hh][:])
                    else:
                        nc.vector.tensor_copy(rs[:], av_t[hh][:])
                    tiles.append(rs)
                    # den row spread: den_sp[p, k*8+c] = den[i = c*128+p]
                    nc.sync.dma_start(den_sp[:, k * 8:(k + 1) * 8],
                                      rs[64:65, :])
                rawst[(pair, ibh)] = tiles
                # den' = den + g ; rb = 1/den' ; zb = g*rb
                gsl = g_nat[:, pair, :, ibh * 8:(ibh + 1) * 8]
                for hh in range(2):
                    k = k0 + hh
                    nc.vector.tensor_tensor(
                        out=den_sp[:, k * 8:(k + 1) * 8],
                        in0=den_sp[:, k * 8:(k + 1) * 8],
                        in1=gsl[:, hh, :], op=mybir.AluOpType.add,
                    )
                with nc.allow_low_precision(reason="bf16 recip validated in sim"):
                    nc.vector.reciprocal(
                        recip_sp[:, k0 * 8:(k0 + 2) * 8],
                        den_sp[:, k0 * 8:(k0 + 2) * 8],
                    )
                for hh in range(2):
                    k = k0 + hh
                    nc.vector.tensor_tensor(
                        out=z_sp[:, k * 8:(k + 1) * 8],
                        in0=recip_sp[:, k * 8:(k + 1) * 8],
                        in1=gsl[:, hh, :], op=mybir.AluOpType.mult,
                    )
                    # rows to DRAM in i-order, then broadcast down 64 partitions
                    nc.sync.dma_start(dscr[0, pair, ibh, hh, :],
                                      recip_sp[:, k * 8:(k + 1) * 8])
                    nc.sync.dma_start(dscr[1, pair, ibh, hh, :],
                                      z_sp[:, k * 8:(k + 1) * 8])
                    for ch in range(2):
                        nc.sync.dma_start(
                            bc[:, hh, ch, :],
                            dscr[ch, pair, ibh, hh, :].partition_broadcast(64),
                        )

            def emit_merge(pair, ibh):
                """osT2 = rb*raw + zb*wm for one (pair, i-half)."""
                i0 = ibh * 1024
                bc = bcasts[(pair, ibh)]
                for hh in range(2):
                    p0 = hh * 64
                    t1 = mrgp.tile([64, 1024], BF, name="t1", tag="t1")
                    t2 = mrgp.tile([64, 1024], BF, name="t2", tag="t2")
                    nc.vector.tensor_tensor(
                        out=t1[:], in0=rawst[(pair, ibh)][hh][0:64, :],
                        in1=bc[:, hh, 0, :],
                        op=mybir.AluOpType.mult)
                    nc.gpsimd.tensor_tensor(
                        out=t2[:], in0=wm2[:, pair, hh, i0:i0 + 1024],
                        in1=bc[:, hh, 1, :],
                        op=mybir.AluOpType.mult)
                    nc.vector.tensor_tensor(
                        out=osT2[p0:p0 + 64, pair, i0:i0 + 1024],
                        in0=t1[:], in1=t2[:], op=mybir.AluOpType.add)

            # ---- phase 3: attention ----
            with (
                tc.tile_pool(name="ps_s", bufs=2, space="PSUM") as ps_s,
                tc.tile_pool(name="ps_av", bufs=1, space="PSUM") as ps_av,
            ):
                AV_LAG = 2
                for pair in range(PAIRS):
                    for ibh in range(2):
                        i0 = ibh * 1024
                        blk = pair * 2 + ibh
                        av_t = [ps_av.tile([65, 1024], FP, name=f"av{hh}",
                                           tag=f"av{hh}") for hh in range(2)]
                        pend = []

                        def flush_av(av_t=av_t, pair=pair, pend=pend):
                            jcp, e_t = pend.pop(0)
                            for hh in range(2):
                                for sb in range(2):
                                    nc.tensor.matmul(
                                        av_t[hh][:, sb * 512:(sb + 1) * 512],
                                        v2[:, pair, jcp, :, hh, 0:65],
                                        e_t[hh][:, sb],
                                        start=(jcp == 0), stop=(jcp == 7),
                                        perf_mode=DR,
                                    )

                        for jcp in range(8):
                            if jcp == 2 and blk > 0:
                                emit_merge((blk - 1) // 2, (blk - 1) % 2)
                            e_t = [epool.tile([128, 2, 2, 512], F8,
                                              name=f"e{hh}", tag=f"e{hh}")
                                   for hh in range(2)]
                            for kt in range(2):
                                jc = jcp * 2 + kt
                                # does this jc hold the block's diagonal?
                                hasd = ibh * 8 <= jc < (ibh + 1) * 8
                                for hh in range(2):
                                    s_t = ps_s.tile([128, 1024], FP,
                                                    name="s", tag="s")
                                    for sb in range(2):
                                        dlast = hasd and sb == 1
                                        nc.tensor.matmul(
                                            s_t[:, sb * 512:(sb + 1) * 512],
                                            wf8[:, pair, hh, :,
                                                jc * 128:(jc + 1) * 128],
                                            wf8[:, pair, hh, :,
                                                i0 + sb * 512:i0 + (sb + 1) * 512],
                                            start=True, stop=not dlast,
                                            perf_mode=DR,
                                            tile_position=(0, 0),
                                            skip_group_check=True,
                                        )
                                    if hasd:
                                        c0 = jc * 128 - i0
                                        nc.tensor.matmul(
                                            s_t[:, c0:c0 + 128],
                                            id128[:], negBI[:],
                                            start=False, stop=True,
                                            skip_group_check=True,
                                        )
                                    # exp -> fp8 E  [128, 1024] unit
                                    eng = sched[expi[0]]
                                    expi[0] += 1
                                    edst = e_t[hh][:, :, kt, :]
                                    if eng == "A":
                                        nc.scalar.activation(
                                            edst, s_t[:].rearrange(
                                                "p (a b) -> p a b", a=2),
                                            EXPF, scale=SCALE, bias=expb[:])
                                    else:
                                        nc.vector.tensor_scalar(
                                            out=edst.bitcast(U8),
                                            in0=s_t[:].rearrange(
                                                "p (a b) -> p a b", a=2),
                                            scalar1=float(A8), scalar2=float(B8),
                                            op0=mybir.AluOpType.mult,
                                            op1=mybir.AluOpType.add)
                            pend.append((jcp, e_t))
                            if len(pend) > AV_LAG:
                                flush_av()
                        while pend:
                            flush_av()
                        emit_block_finish(pair, ibh, av_t)

            # ---- tail: last merge + output projection ----
            with tc.tile_pool(name="ps_y", bufs=3, space="PSUM") as ps_y:
                emit_merge(1, 1)
                for ic in range(16):
                    y_ps = ps_y.tile([128, 1024], FP, name="yp", tag="y")
                    for sb in range(2):
                        for pair in range(PAIRS):
                            nc.tensor.matmul(
                                y_ps[:, sb * 512:(sb + 1) * 512],
                                osT2[:, pair, ic * 128:(ic + 1) * 128],
                                wout_sb[:, pair, sb * 512:(sb + 1) * 512],
                                start=(pair == 0), stop=(pair == PAIRS - 1),
                            )
                    y_sb = ysbp.tile([128, 1024], BF, name="ysb", tag="ysb")
                    if ic % 2 == 0:
                        nc.scalar.copy(y_sb[:], y_ps[:])
                    else:
                        nc.vector.tensor_copy(y_sb[:], y_ps[:])
                    yeng = nc.sync if ic % 2 == 0 else nc.scalar
                    yeng.dma_start(y[ic * 128:(ic + 1) * 128, :], y_sb[:])

    return nc


def get_program():
    if "nc" not in _program_cache:
        _program_cache["nc"] = build_program()
    return _program_cache["nc"]


def make_in_maps(x, mask, Wqkv, Wout):
    xT_b = [np.ascontiguousarray(np.asarray(x)[b].T).astype(BF_NP) for b in range(2)]
    wq_bf = np.asarray(Wqkv).astype(BF_NP)
    wo_bf = np.asarray(Wout).astype(BF_NP)
    mask_np = np.asarray(mask)
    in_maps = []
    for c in range(8):
        b, hg = c // 4, c % 4
        ec = slice(hg * EC, (hg + 1) * EC)
        # pre-transposed layouts: wq [128, kc, 256], wout [128, pc, 1024]
        wq_l = np.ascontiguousarray(
            wq_bf[:, ec].reshape(KC, 128, EC).transpose(1, 0, 2))
        wo_l = np.ascontiguousarray(
            wo_bf[ec, :].reshape(PAIRS, 128, DIM).transpose(1, 0, 2))
        in_maps.append({
            "xT": xT_b[b],
            "wqkv": wq_l,
            "wout": wo_l,
            "mask": np.ascontiguousarray(mask_np[b]),
            "mf": np.ascontiguousarray((1 - mask_np[b]).astype(BF_NP)),
        })
    return in_maps


def assemble(results, bout):
    y = np.stack([
        sum(results[b * 4 + g]["y"].astype(np.float32) for g in range(4))
        for b in range(2)
    ])
    return (y + np.asarray(bout)[None, None, :]).astype(np.float32)


def kernel(x, mask, Wqkv, Wout, bout):
    _install_bir_legalizer()
    nc = get_program()
    in_maps = make_in_maps(x, mask, Wqkv, Wout)
    res = run_bass_kernel_spmd(nc, in_maps, core_ids=list(range(8)))
    return assemble(res.results, bout)


if __name__ == "__main__":
    nc = build_program()
    print("program built OK")


# revision 12
# speedup vs baseline: 1.1169x; 1.1169x over previous
"""Trainium2 Bass kernel for CRATE-style subspace attention (nn_Attention_37091337568712).

Reference computation (fp32):
    w = x @ Wqkv                    # (b, n, 1024), shared q=k=v projection
    S = (w @ w^T) * d^-0.5          # per head, (b, h, n, n)
    attn = softmax(S, axis=-1) * (1 - mask[:, None, None, :])
    out = attn @ w ; y = out @ Wout + bout

Sharding: 8 cores = 2 batches x 4 head-groups (4 heads each, 2 pairs of 2).

fp8 design (vs the 204us bf16 baseline):
  - The gram-matrix diagonal S_ii = |w_i|^2/8 ~ 8..15 dominates softmax and
    cannot survive fp8 E. Diagonal extraction: S' = S - 192*I via a small
    accumulating identity matmul, so exp(S') underflows fp8 to exact 0 on the
    diagonal; the exact diagonal term g_i = exp(SCALE*s_ii - SH*ln2) is
    computed separately (w^2 ones-matmul + one ACT exp on a spread layout)
    and merged into numerator/denominator at scale time.
  - With the diagonal gone, logits fit fp8: S via DoubleRow fp8 matmuls
    (w as [32, 2kt, .] row-banded per head, 2 heads co-executing), E in fp8
    (ACT: true exp w/ -SH*ln2 bias; DVE/Pool: Schraudolph uint8 bit trick,
    negative bits saturate to +0), V' in fp8, AV as DoubleRow over jc pairs.
    PE attention work drops ~2.6x; exp is split over ACT+DVE+Pool.
  - softmax denominator via unmasked ones columns in the AV stationary (M=65).
  - numerator merge: osT = rb*raw + zb*wm (rb=1/den', zb=g/den' broadcast
    rows; wm = masked w kept per-head at partitions 0..63).
  - numpy-sim of this exact pipeline: rel err ~5.5e-3 (gate 2e-2).
"""

import sys

if "/opt/trn_rl_repo" not in sys.path:
    sys.path.insert(0, "/opt/trn_rl_repo")

import numpy as np
import ml_dtypes

import concourse.bass as bass
import concourse.mybir as mybir
from concourse import masks
from concourse.bass_utils import run_bass_kernel_spmd
from concourse.tile import TileContext

FP = mybir.dt.float32
I32 = mybir.dt.int32
I16 = mybir.dt.int16
U8 = mybir.dt.uint8
BF = mybir.dt.bfloat16
F8 = mybir.dt.float8e4
BF_NP = ml_dtypes.bfloat16
DR = mybir.MatmulPerfMode.DoubleRow


def _split_multiwaits(bir_json: bytes) -> bytes:
    """Split multi-wait instructions for this container's walrus (single
    sync wait per instruction)."""
    import json

    bir = json.loads(bir_json)
    changed = False
    for fn in bir.get("functions", []):
        for bb in fn.get("blocks", []):
            insts = bb.get("instructions")
            if insts is None:
                continue
            new_insts = []
            for ins in insts:
                si = ins.get("sync_info")
                waits = si.get("on_wait") if si else None
                if waits and len(waits) > 1:
                    changed = True
                    for wi, w in enumerate(waits[:-1]):
                        new_insts.append({
                            "name": f"{ins['name']}_w{wi}",
                            "opcode": "EventSemaphore",
                            "engine": ins["engine"],
                            "ins": [],
                            "outs": [],
                            "debug": ins.get("debug", 0),
                            "sync_info": {"on_wait": [w], "on_update": []},
                        })
                    si["on_wait"] = [waits[-1]]
                new_insts.append(ins)
            bb["instructions"] = new_insts
    if not changed:
        return bir_json
    return json.dumps(bir).encode()


def _install_bir_legalizer():
    from concourse import bass2jax, bass_utils

    if getattr(bass2jax, "_multiwait_legalizer_installed", False):
        return
    orig = bass_utils.compile_bir_kernel

    def wrapped(bir_json, tmpdir, neff_name="file.neff"):
        try:
            return orig(_split_multiwaits(bytes(bir_json)), tmpdir, neff_name)
        except BaseException as e:
            import subprocess, traceback
            try:
                with open("/tmp/bass_compile_err.txt", "w") as f:
                    traceback.print_exc(file=f)
                    ee = e
                    while ee is not None:
                        if isinstance(ee, subprocess.CalledProcessError):
                            out = ee.stdout or ""
                            if isinstance(out, bytes):
                                out = out.decode(errors="replace")
                            f.write("\n==WALRUS STDOUT (tail)==\n" + out[-12000:])
                        ee = ee.__cause__ or ee.__context__
            except Exception:
                pass
            raise

    bass2jax.compile_bir_kernel = wrapped
    bass2jax._multiwait_legalizer_installed = True


N = 2048          # sequence length
DIM = 1024        # model dim
DH = 64           # head dim
EC = 256          # local inner columns (4 heads)
KC = DIM // 128   # 8 contraction chunks for the projection
PAIRS = 2         # head pairs per core (2 heads stacked on 128 partitions)
SCALE = DH ** -0.5
LN2 = float(np.log(2.0))

SH = 2.0          # global log2 shift of E (softmax-invariant)
BDIAG = 192.0     # diagonal extraction; exp(S'-ii) underflows fp8 to 0
C8 = 0.0434       # Schraudolph tuning constant
A8 = 8.0 * (1.0 / LN2) * SCALE        # uint8 Schraudolph: bits = A8*S + B8
B8 = 8.0 * (7.0 - C8 - SH)
C8P = 0.0434      # Pool variant (retune if Pool rounding differs)
B8P = 8.0 * (7.0 - C8P - SH)

EXPF = mybir.ActivationFunctionType.Exp
COPYF = mybir.ActivationFunctionType.Copy

# exp engine split: 'A' = ACT true exp->fp8, 'D' = DVE Schraudolph uint8,
# 'P' = Pool Schraudolph uint8. POOL_EXP gate for bring-up.
POOL_EXP = True
ACT_FP8_OUT = True

_program_cache = {}


def _exp_schedule():
    """Greedy finish-time assignment of the 128 exp units to ACT/DVE.
    (GPSIMD cannot read PSUM, so Pool does SBUF-side work instead.)"""
    cost = {"A": 996.0, "D": 1192.0}
    load = {"A": 7000.0, "D": 15000.0}  # fixed-work seeds
    sched = []
    for _ in range(128):
        e = min(cost, key=lambda k: load[k] + cost[k])
        sched.append(e)
        load[e] += cost[e]
    return sched


def build_program():
    nc = bass.Bass()

    xT = nc.declare_dram_parameter("xT", [DIM, N], BF, isOutput=False)
    wqkv = nc.declare_dram_parameter("wqkv", [128, KC, EC], BF, isOutput=False)
    wout = nc.declare_dram_parameter("wout", [128, PAIRS, DIM], BF, isOutput=False)
    mask_d = nc.declare_dram_parameter("mask", [N], I32, isOutput=False)
    mf_d = nc.declare_dram_parameter("mf", [N], BF, isOutput=False)
    y = nc.declare_dram_parameter("y", [N, DIM], BF, isOutput=True)
    # DRAM scratch: broadcast rows (ch: 0=rb, 1=zb) and the fp8 w bounce
    dscr = nc.declare_dram_parameter("dscr", [2, PAIRS, 2, 2, N // 2], BF,
                                     isOutput=True)
    gdr = nc.declare_dram_parameter("gdr", [PAIRS, 2, N], BF, isOutput=True)

    sched = _exp_schedule()
    expi = [0]  # rotating index into sched

    with TileContext(nc) as tc:
        with (
            tc.tile_pool(name="const", bufs=1) as constp,
            tc.tile_pool(name="wts", bufs=1) as wts,
            tc.tile_pool(name="persist", bufs=1) as persist,
            tc.tile_pool(name="xin", bufs=5) as xin,
            tc.tile_pool(name="epool", bufs=6) as epool,
            tc.tile_pool(name="rawp", bufs=3) as rawp,
            tc.tile_pool(name="trpool", bufs=2) as trpool,
            tc.tile_pool(name="mrgp", bufs=3) as mrgp,
            tc.tile_pool(name="ysb", bufs=2) as ysbp,
            tc.tile_pool(name="bcp", bufs=2) as bcp,
        ):
            # ---- weights + input stream first so the projection starts ASAP
            wq_sb = wts.tile([128, KC, EC], BF)
            nc.sync.dma_start(wq_sb[:], wqkv[:])
            xts = []
            for kc in range(KC):
                xt = xin.tile([128, N], BF, name="xt")
                nc.sync.dma_start(xt[:], xT[kc * 128:(kc + 1) * 128, :])
                xts.append(xt)

            # ---- small inputs + mask broadcast on the scalar queue ----
            mask_i = constp.tile([16, 128], I32)
            nc.scalar.dma_start(mask_i[:], mask_d.rearrange("(a b) -> a b", a=16))
            mbc = persist.tile([128, N], BF)       # (1-m) broadcast over rows
            nc.scalar.dma_start(mbc[:], mf_d[:].partition_broadcast(128))
            wout_sb = wts.tile([128, PAIRS, DIM], BF)
            nc.scalar.dma_start(wout_sb[:], wout[:])

            # ---- constants ----
            ident_f = constp.tile([16, 16], FP)
            masks.make_identity(nc, ident_f[:])
            id128 = constp.tile([128, 128], BF)
            masks.make_identity(nc, id128[:])
            negBI = constp.tile([128, 128], BF)
            nc.vector.tensor_scalar_mul(negBI[:], id128[:], -BDIAG)
            expb = constp.tile([128, 1], FP)       # exp bias: -SH*ln2
            nc.gpsimd.memset(expb[:], -SH * LN2)
            hones = constp.tile([128, 2], BF)      # head-membership ones
            nc.gpsimd.memset(hones[:], 0.0)
            nc.gpsimd.memset(hones[0:64, 0:1], 1.0)
            nc.gpsimd.memset(hones[64:128, 1:2], 1.0)
            mask_f = constp.tile([16, 128], FP)
            nc.vector.tensor_scalar(
                out=mask_f[:], in0=mask_i[:], scalar1=-1.0, scalar2=1.0,
                op0=mybir.AluOpType.mult, op1=mybir.AluOpType.add,
            )

            # ---- persistent big tiles ----
            wT2 = persist.tile([128, PAIRS, N], BF)        # [d2, pair, i]
            wm2 = persist.tile([64, PAIRS, 2, N], BF)      # masked w, head rows 0:64
            wsq = persist.tile([128, PAIRS, N], BF)        # w*w
            v2 = persist.tile([128, PAIRS, 8, 2, 2, 72], F8)  # [j, pair, jcp, kt, hh, d72]
            osT2 = persist.tile([128, PAIRS, N], BF)       # merged attn out
            maskc = persist.tile([128, 16], FP)            # (1-mask) [j%128, jc]
            g_sp = persist.tile([128, PAIRS, 16, 2], BF)   # matmul-layout exp(diag)
            g_nat = persist.tile([128, PAIRS, 2, 16], BF)  # natural spread layout
            den_sp = persist.tile([128, 64], BF)
            recip_sp = persist.tile([128, 64], BF)
            z_sp = persist.tile([128, 64], BF)


            # ---- phase 1: projection ----
            with tc.tile_pool(name="ps_proj", bufs=1, space="PSUM") as ps_proj:
                proj_ps = [ps_proj.tile([128, 1024], FP, name=f"proj{t}", tag=f"proj{t}")
                           for t in range(4)]
                for kc in range(KC):
                    for pair in range(PAIRS):
                        for nb in range(2):
                            for sb in range(2):
                                nc.tensor.matmul(
                                    proj_ps[pair * 2 + nb][:, sb * 512:(sb + 1) * 512],
                                    wq_sb[:, kc, pair * 128:(pair + 1) * 128],
                                    xts[kc][:, nb * 1024 + sb * 512:
                                            nb * 1024 + (sb + 1) * 512],
                                    start=(kc == 0), stop=(kc == KC - 1),
                                )
                for pair in range(PAIRS):
                    for nb in range(2):
                        eng = nc.scalar if nb == 0 else nc.vector
                        eng_copy = (nc.scalar.copy if nb == 0
                                    else nc.vector.tensor_copy)
                        eng_copy(wT2[:, pair, nb * 1024:(nb + 1) * 1024],
                                 proj_ps[pair * 2 + nb][:])

            # ---- mask layout transpose ----
            with tc.tile_pool(name="ps_tr", bufs=1, space="PSUM") as ps_tr:
                mt_ps = ps_tr.tile([128, 16], FP, tag="trm")
                nc.tensor.transpose(mt_ps[:], mask_f[:], ident_f[:])
                nc.vector.tensor_copy(maskc[:], mt_ps[:])

            # ---- per-pair prep: wm/wsq/cast/bounce/transpose/V' ----
            nc.vector.memset(v2[:, :, :, :, :, 64], 1.0)
            for pair in range(PAIRS):
                # masked w, per-head at partitions 0:63 (merge operand)
                for hh in range(2):
                    nc.vector.tensor_tensor(
                        out=wm2[:, pair, hh, :],
                        in0=wT2[hh * 64:(hh + 1) * 64, pair, :],
                        in1=mbc[hh * 64:(hh + 1) * 64, :],
                        op=mybir.AluOpType.mult,
                    )
                # w^2 for the diagonal
                nc.vector.tensor_tensor(
                    out=wsq[:, pair, :], in0=wT2[:, pair, :],
                    in1=wT2[:, pair, :], op=mybir.AluOpType.mult,
                )
                # V' via one batched XBAR transpose + masked fp8 build
                tr2 = trpool.tile([128, 16, 128], BF, name="tr2", tag="tr2")
                nc.sync.dma_start_transpose(tr2[:], wT2[:, pair, :])
                for jc in range(16):
                    vdst = v2[:, pair, jc // 2, jc % 2, :, 0:64]
                    vsrc = tr2[:, jc, :].rearrange("p (h x) -> p h x", h=2)
                    nc.vector.tensor_scalar_mul(vdst, vsrc, maskc[:, jc:jc + 1])

            # ---- diagonal g: ones-matmuls on wsq + one ACT exp per pair ----
            with tc.tile_pool(name="ps_g", bufs=1, space="PSUM") as ps_g:
                for pair in range(PAIRS):
                    g_ps = ps_g.tile([128, 16, 2], FP, name="gps", tag="gps")
                    for c in range(16):
                        nc.tensor.matmul(
                            g_ps[:, c, :],
                            wsq[:, pair, c * 128:(c + 1) * 128],
                            hones[:],
                            start=True, stop=True,
                        )
                    nc.scalar.activation(g_sp[:, pair, :, :], g_ps[:],
                                         EXPF, scale=SCALE, bias=expb[:])
                    for hh in range(2):
                        nc.scalar.dma_start(
                            gdr[pair, hh, :].rearrange("(c p) -> p c", p=128),
                            g_sp[:, pair, :, hh])
                        for ibh in range(2):
                            nc.scalar.dma_start(
                                g_nat[:, pair, hh, ibh * 8:(ibh + 1) * 8],
                                gdr[pair, hh, ibh * 1024:(ibh + 1) * 1024])

            # ---- helper emitters ----
            rawst = {}
            bcasts = {}

            def emit_block_finish(pair, ibh, av_t):
                k0 = (pair * 2 + ibh) * 2
                bc = bcp.tile([64, 2, 2, N // 2], BF, name="bc", tag="bc")
                bcasts[(pair, ibh)] = bc
                tiles = []
                for hh in range(2):
                    k = k0 + hh
                    rs = rawp.tile([65, 1024], BF, name=f"rawst{hh}",
                                   tag=f"rawst{hh}")
                    if hh == 0:
                        nc.scalar.copy(rs[:], av_t[hh][:])
                    else:
                        nc.vector.tensor_copy(rs[:], av_t[hh][:])
                    tiles.append(rs)
                    # den row spread: den_sp[p, k*8+c] = den[i = c*128+p]
                    nc.sync.dma_start(den_sp[:, k * 8:(k + 1) * 8],
                                      rs[64:65, :])
                rawst[(pair, ibh)] = tiles
                # den' = den + g ; rb = 1/den' ; zb = g*rb
                gsl = g_nat[:, pair, :, ibh * 8:(ibh + 1) * 8]
                for hh in range(2):
                    k = k0 + hh
                    nc.vector.tensor_tensor(
                        out=den_sp[:, k * 8:(k + 1) * 8],
                        in0=den_sp[:, k * 8:(k + 1) * 8],
                        in1=gsl[:, hh, :], op=mybir.AluOpType.add,
                    )
                with nc.allow_low_precision(reason="bf16 recip validated in sim"):
                    nc.vector.reciprocal(
                        recip_sp[:, k0 * 8:(k0 + 2) * 8],
                        den_sp[:, k0 * 8:(k0 + 2) * 8],
                    )
                for hh in range(2):
                    k = k0 + hh
                    nc.vector.tensor_tensor(
                        out=z_sp[:, k * 8:(k + 1) * 8],
                        in0=recip_sp[:, k * 8:(k + 1) * 8],
                        in1=gsl[:, hh, :], op=mybir.AluOpType.mult,
                    )
                    # rows to DRAM in i-order, then broadcast down 64 partitions
                    nc.sync.dma_start(dscr[0, pair, ibh, hh, :],
                                      recip_sp[:, k * 8:(k + 1) * 8])
                    nc.sync.dma_start(dscr[1, pair, ibh, hh, :],
                                      z_sp[:, k * 8:(k + 1) * 8])
                    for ch in range(2):
                        nc.sync.dma_start(
                            bc[:, hh, ch, :],
                            dscr[ch, pair, ibh, hh, :].partition_broadcast(64),
                        )

            def emit_merge(pair, ibh):
                """osT2 = rb*raw + zb*wm for one (pair, i-half)."""
                i0 = ibh * 1024
                bc = bcasts[(pair, ibh)]
                for hh in range(2):
                    p0 = hh * 64
                    t1 = mrgp.tile([64, 1024], BF, name="t1", tag="t1")
                    t2 = mrgp.tile([64, 1024], BF, name="t2", tag="t2")
                    nc.vector.tensor_tensor(
                        out=t1[:], in0=rawst[(pair, ibh)][hh][0:64, :],
                        in1=bc[:, hh, 0, :],
                        op=mybir.AluOpType.mult)
                    nc.gpsimd.tensor_tensor(
                        out=t2[:], in0=wm2[:, pair, hh, i0:i0 + 1024],
                        in1=bc[:, hh, 1, :],
                        op=mybir.AluOpType.mult)
                    nc.vector.tensor_tensor(
                        out=osT2[p0:p0 + 64, pair, i0:i0 + 1024],
                        in0=t1[:], in1=t2[:], op=mybir.AluOpType.add)

            # ---- phase 3: attention ----
            with (
                tc.tile_pool(name="ps_s", bufs=2, space="PSUM") as ps_s,
                tc.tile_pool(name="ps_av", bufs=1, space="PSUM") as ps_av,
            ):
                AV_LAG = 2
                for pair in range(PAIRS):
                    for ibh in range(2):
                        i0 = ibh * 1024
                        blk = pair * 2 + ibh
                        av_t = [ps_av.tile([65, 1024], FP, name=f"av{hh}",
                                           tag=f"av{hh}") for hh in range(2)]
                        pend = []

                        def flush_av(av_t=av_t, pair=pair, pend=pend):
                            jcp, e_t = pend.pop(0)
                            for hh in range(2):
                                for sb in range(2):
                                    nc.tensor.matmul(
                                        av_t[hh][:, sb * 512:(sb + 1) * 512],
                                        v2[:, pair, jcp, :, hh, 0:65],
                                        e_t[hh][:, sb].rearrange(
                                            "p n kt -> p kt n"),
                                        start=(jcp == 0), stop=(jcp == 7),
                                        perf_mode=DR,
                                    )

                        for jcp in range(8):
                            if jcp == 2 and blk > 0:
                                emit_merge((blk - 1) // 2, (blk - 1) % 2)
                            e_t = [epool.tile([128, 2, 512, 2], F8,
                                              name=f"e{hh}", tag=f"e{hh}")
                                   for hh in range(2)]
                            for kt in range(2):
                                jc = jcp * 2 + kt
                                # does this jc hold the block's diagonal?
                                hasd = ibh * 8 <= jc < (ibh + 1) * 8
                                for hh in range(2):
                                    h64 = hh * 64
                                    s_t = ps_s.tile([128, 1024], FP,
                                                    name="s", tag="s")
                                    for sb in range(2):
                                        dlast = hasd and sb == 1
                                        nc.tensor.matmul(
                                            s_t[:, sb * 512:(sb + 1) * 512],
                                            wT2[h64:h64 + 64, pair,
                                                jc * 128:(jc + 1) * 128],
                                            wT2[h64:h64 + 64, pair,
                                                i0 + sb * 512:i0 + (sb + 1) * 512],
                                            start=True, stop=not dlast,
                                            tile_position=(h64, 0),
                                            skip_group_check=True,
                                        )
                                    if hasd:
                                        c0 = jc * 128 - i0
                                        nc.tensor.matmul(
                                            s_t[:, c0:c0 + 128],
                                            id128[:], negBI[:],
                                            start=False, stop=True,
                                            skip_group_check=True,
                                        )
                                    # exp -> fp8 E  [128, 1024] unit
                                    eng = sched[expi[0]]
                                    expi[0] += 1
                                    edst = e_t[hh][:, :, :, kt]
                                    if eng == "A":
                                        nc.scalar.activation(
                                            edst, s_t[:].rearrange(
                                                "p (a b) -> p a b", a=2),
                                            EXPF, scale=SCALE, bias=expb[:])
                                    else:
                                        nc.vector.tensor_scalar(
                                            out=edst.bitcast(U8),
                                            in0=s_t[:].rearrange(
                                                "p (a b) -> p a b", a=2),
                                            scalar1=float(A8), scalar2=float(B8),
                                            op0=mybir.AluOpType.mult,
                                            op1=mybir.AluOpType.add)
                            pend.append((jcp, e_t))
                            if len(pend) > AV_LAG:
                                flush_av()
                        while pend:
                            flush_av()
                        emit_block_finish(pair, ibh, av_t)

            # ---- tail: last merge + output projection ----
            with tc.tile_pool(name="ps_y", bufs=3, space="PSUM") as ps_y:
                emit_merge(1, 1)
                for ic in range(16):
                    y_ps = ps_y.tile([128, 1024], FP, name="yp", tag="y")
                    for sb in range(2):
                        for pair in range(PAIRS):
                            nc.tensor.matmul(
                                y_ps[:, sb * 512:(sb + 1) * 512],
                                osT2[:, pair, ic * 128:(ic + 1) * 128],
                                wout_sb[:, pair, sb * 512:(sb + 1) * 512],
                                start=(pair == 0), stop=(pair == PAIRS - 1),
                            )
                    y_sb = ysbp.tile([128, 1024], BF, name="ysb", tag="ysb")
                    if ic % 2 == 0:
                        nc.scalar.copy(y_sb[:], y_ps[:])
                    else:
                        nc.vector.tensor_copy(y_sb[:], y_ps[:])
                    yeng = nc.sync if ic % 2 == 0 else nc.scalar
                    yeng.dma_start(y[ic * 128:(ic + 1) * 128, :], y_sb[:])

    return nc


def get_program():
    if "nc" not in _program_cache:
        _program_cache["nc"] = build_program()
    return _program_cache["nc"]


def make_in_maps(x, mask, Wqkv, Wout):
    xT_b = [np.ascontiguousarray(np.asarray(x)[b].T).astype(BF_NP) for b in range(2)]
    wq_bf = np.asarray(Wqkv).astype(BF_NP)
    wo_bf = np.asarray(Wout).astype(BF_NP)
    mask_np = np.asarray(mask)
    in_maps = []
    for c in range(8):
        b, hg = c // 4, c % 4
        ec = slice(hg * EC, (hg + 1) * EC)
        # pre-transposed layouts: wq [128, kc, 256], wout [128, pc, 1024]
        wq_l = np.ascontiguousarray(
            wq_bf[:, ec].reshape(KC, 128, EC).transpose(1, 0, 2))
        wo_l = np.ascontiguousarray(
            wo_bf[ec, :].reshape(PAIRS, 128, DIM).transpose(1, 0, 2))
        in_maps.append({
            "xT": xT_b[b],
            "wqkv": wq_l,
            "wout": wo_l,
            "mask": np.ascontiguousarray(mask_np[b]),
            "mf": np.ascontiguousarray((1 - mask_np[b]).astype(BF_NP)),
        })
    return in_maps


def assemble(results, bout):
    y = np.stack([
        sum(results[b * 4 + g]["y"].astype(np.float32) for g in range(4))
        for b in range(2)
    ])
    return (y + np.asarray(bout)[None, None, :]).astype(np.float32)


def kernel(x, mask, Wqkv, Wout, bout):
    _install_bir_legalizer()
    nc = get_program()
    in_maps = make_in_maps(x, mask, Wqkv, Wout)
    res = run_bass_kernel_spmd(nc, in_maps, core_ids=list(range(8)))
    return assemble(res.results, bout)


if __name__ == "__main__":
    nc = build_program()
    print("program built OK")
